# revision 1
# baseline (speedup 1.0000x reference)
"""MultiHeadGAT layer on 8 trn2 NeuronCores, data-parallel over batch.

Per core (one batch element):
  Wh = h @ W                                  [1024, 512]  (heads=8, fo=64)
  e_src[h,i], e_dst[h,i] from E = h @ (W @ A)  (WA precomputed on host)
  scores_T[j,i] = leaky_relu(e_src[i] + e_dst[j])   (transposed: j on partitions)
  P = exp(scores_T) * adjT    where exp(leaky(s)) == max(exp(s), exp(0.2 s))
  out[i, h*64+f] = (P.T @ Wh_h)[i,f] / sum_j P[j,i]

AV matmul in transposed orientation: out_T[f,i] = sum_j Wh[j,f]*P[j,i], with a
ones column appended to the lhsT so row 64 of the accumulator is the softmax
denominator.  Engine budget: ACT does the two exps per tile (bias/scale fold
the e_dst add and the 0.2 slope), DVE does max, gpsimd+DVE split the adjacency
mask multiply (adjT kept in bf16 - exact for 0/1 - produced by DMA transpose),
PE does the matmuls fp32.
"""
import sys

sys.path.insert(0, "/opt/trn_rl_repo")

import numpy as np

import concourse.bass as bass
import concourse.mybir as mybir
import concourse.tile as tile
from concourse.bass_utils import run_bass_kernel_spmd
from concourse.masks import make_identity

F32 = mybir.dt.float32
BF16 = mybir.dt.bfloat16
I32 = mybir.dt.int32
AF = mybir.ActivationFunctionType

N_CORES = 8
N = 1024
NB = 8          # row blocks of 128
FIN = 256
KT = 2          # FIN / 128
FO = 512        # heads * fo
H = 8
FOH = 64
ALPHA = 0.2

# tiles whose mask-multiply / max run on gpsimd instead of DVE
GP_MASK_JB = 0  # jb < GP_MASK_JB -> gpsimd handles the mask for that tile

_MAX_SYNC_WAITS = 1


def _split_sync_waits(nc, max_waits=_MAX_SYNC_WAITS):
    """This walrus build rejects instructions carrying more than one sync
    wait; hoist extras onto NOPs inserted just before, on the same engine."""
    uid = 0
    for f in nc.m.functions:
        for bb in f.blocks:
            out = []
            for inst in bb.instructions:
                si = getattr(inst, "sync_info", None)
                if si is not None and si.on_wait and len(si.on_wait) > max_waits:
                    waits = list(si.on_wait)
                    keep = waits[-max_waits:]
                    extra = waits[:-max_waits]
                    si.on_wait.clear()
                    si.on_wait.extend(keep)
                    while extra:
                        chunk, extra = extra[:max_waits], extra[max_waits:]
                        nop = mybir.InstNoOp(
                            name=f"waitsplit-{uid}",
                            engine=inst.engine,
                            sync_info=mybir.SyncInfo(
                                on_wait=list(chunk), on_update=[]
                            ),
                            bass_nofuse=True,
                        )
                        uid += 1
                        out.append(nop)
                out.append(inst)
            bb.instructions[:] = out


def build_nc(split=True):
    nc = bass.Bass()
    h_d = nc.declare_dram_parameter("h", [N, FIN], F32, isOutput=False)
    adj_d = nc.declare_dram_parameter("adj", [N, N], I32, isOutput=False)
    w_d = nc.declare_dram_parameter("W", [FIN, FO], F32, isOutput=False)
    wa_d = nc.declare_dram_parameter("WA", [FIN, 2 * H], F32, isOutput=False)
    out_d = nc.declare_dram_parameter("out", [N, FO], F32, isOutput=True)

    with tile.TileContext(nc) as tc:
        with (
            tc.tile_pool(name="const", bufs=1) as const,
            tc.tile_pool(name="persist", bufs=1) as persist,
            tc.tile_pool(name="ld", bufs=4) as ld,
            tc.tile_pool(name="x1p", bufs=8) as x1p,
            tc.tile_pool(name="x2p", bufs=5) as x2p,
            tc.tile_pool(name="epi", bufs=3) as epi,
            tc.tile_pool(name="psS", bufs=3, space="PSUM") as psS,
            tc.tile_pool(name="psAcc", bufs=2, space="PSUM") as psAcc,
        ):
            ident = const.tile([128, 128], F32, tag="ident")
            make_identity(nc, ident[:])

            wk = []
            for k in range(KT):
                t = const.tile([128, FO], F32, tag=f"W{k}", name=f"W{k}")
                nc.sync.dma_start(t[:], w_d[k * 128:(k + 1) * 128, :])
                wk.append(t)
            wa = []
            for k in range(KT):
                t = const.tile([128, 2 * H], F32, tag=f"WA{k}", name=f"WA{k}")
                nc.sync.dma_start(t[:], wa_d[k * 128:(k + 1) * 128, :])
                wa.append(t)

            # ---- hT[k][f128, i] = h[i, k*128+f] ----
            hT = [persist.tile([128, N], F32, tag=f"hT{k}", name=f"hT{k}")
                  for k in range(KT)]
            for ibq in range(2):      # groups of 4 row-blocks
                hts = []
                for i4 in range(4):
                    ib = ibq * 4 + i4
                    ht = ld.tile([128, FIN], F32, tag="hld")
                    nc.sync.dma_start(ht[:], h_d[ib * 128:(ib + 1) * 128, :])
                    hts.append(ht)
                for k in range(KT):
                    tp = psS.tile([128, 512], F32, tag="ps")
                    for i4 in range(4):
                        nc.tensor.transpose(
                            tp[:, i4 * 128:(i4 + 1) * 128],
                            hts[i4][:, k * 128:(k + 1) * 128], ident[:],
                        )
                    nc.vector.tensor_copy(
                        hT[k][:, ibq * 512:(ibq + 1) * 512], tp[:]
                    )

            # ---- Wh_aug[jb][:, hh*65:+64] = (h @ W) block, col hh*65+64 = 1 ----
            wh_aug = [persist.tile([128, H * 65], F32, tag=f"wha{j}", name=f"wha{j}")
                      for j in range(NB)]
            for jb in range(NB):
                ps = psS.tile([128, 512], F32, tag="ps")
                for k in range(KT):
                    nc.tensor.matmul(
                        ps[:], hT[k][:, jb * 128:(jb + 1) * 128], wk[k][:],
                        start=(k == 0), stop=(k == KT - 1),
                    )
                for hh in range(H):
                    nc.vector.tensor_copy(
                        wh_aug[jb][:, hh * 65:hh * 65 + 64],
                        ps[:, hh * 64:(hh + 1) * 64],
                    )
                for hh in range(H):
                    nc.gpsimd.memset(
                        wh_aug[jb][:, hh * 65 + 64:hh * 65 + 65], 1.0
                    )

            # ---- E_T[16, i] = (WA.T @ hT): rows 0..7 e_src, 8..15 e_dst ----
            e_t = const.tile([16, N], F32, tag="eT")
            for c in range(2):
                ps = psS.tile([16, 512], F32, tag="ps")
                for k in range(KT):
                    nc.tensor.matmul(
                        ps[:], wa[k][:], hT[k][:, c * 512:(c + 1) * 512],
                        start=(k == 0), stop=(k == KT - 1),
                    )
                nc.vector.tensor_copy(e_t[:, c * 512:(c + 1) * 512], ps[:])

            # ---- E[jb][p, 16] = E_T[:, jb*128+p]; e_sc = 0.2 * E ----
            e_sb = [persist.tile([128, 16], F32, tag=f"E{j}", name=f"E{j}")
                    for j in range(NB)]
            e_sc = [persist.tile([128, 16], F32, tag=f"Es{j}", name=f"Es{j}")
                    for j in range(NB)]
            for jb in range(NB):
                tp = psS.tile([128, 512], F32, tag="ps")
                nc.tensor.transpose(
                    tp[:, 0:16], e_t[:, jb * 128:(jb + 1) * 128],
                    ident[0:16, 0:16],
                )
                nc.vector.tensor_copy(e_sb[jb][:], tp[:, 0:16])
                nc.vector.tensor_scalar_mul(e_sc[jb][:], tp[:, 0:16], ALPHA)

            # ---- e_srcb[h][p, i] = e_src[h, i] broadcast over partitions.
            # Heads 0-1 via PE selector matmul (low latency, unblocks the main
            # loop); heads 2-7 via DMA log-doubling (no PE cost, latency
            # hidden behind the first heads' compute). ----
            e_srcb = [persist.tile([128, N], F32, tag=f"esb{hh}", name=f"esb{hh}")
                      for hh in range(H)]
            NSEL = 2
            sel = []
            for hh in range(NSEL):
                t = const.tile([16, 128], F32, tag=f"sel{hh}", name=f"sel{hh}")
                nc.gpsimd.memset(t[:], 0.0)
                # t[p, y] = (p == hh) ? 1.0 : 0.0
                nc.gpsimd.affine_select(
                    out=t[:], in_=t[:], pattern=[[0, 128]],
                    compare_op=mybir.AluOpType.not_equal, fill=1.0,
                    base=-hh, channel_multiplier=1,
                )
                sel.append(t)
            for hh in range(NSEL):
                for c in range(2):
                    ps = psS.tile([128, 512], F32, tag="ps")
                    nc.tensor.matmul(
                        ps[:], sel[hh][:], e_t[:, c * 512:(c + 1) * 512],
                        start=True, stop=True,
                    )
                    nc.vector.tensor_copy(
                        e_srcb[hh][:, c * 512:(c + 1) * 512], ps[:]
                    )
            for hh in range(NSEL, H):
                t = e_srcb[hh]
                nc.sync.dma_start(t[0:1, :], e_t[hh:hh + 1, :])
                p = 1
                while p < 128:
                    nc.sync.dma_start(t[p:2 * p, :], t[0:p, :])
                    p *= 2

            # ---- adjT[jb][j128, i] = adj[i, jb*128+j] as bf16 (PE transpose).
            # jb-major so adjT[0] completes first and unblocks the main loop
            # as early as possible. ----
            identb = const.tile([128, 128], BF16, tag="identb")
            nc.vector.tensor_copy(identb[:], ident[:])
            adjT = [persist.tile([128, N], BF16, tag=f"adjT{j}", name=f"adjT{j}")
                    for j in range(NB)]
            adjf = [persist.tile([128, N], BF16, tag=f"adjf{i}", name=f"adjf{i}")
                    for i in range(NB)]
            for ib in range(NB):
                ai = ld.tile([128, N], I32, tag="adji", bufs=3)
                nc.sync.dma_start(ai[:], adj_d[ib * 128:(ib + 1) * 128, :])
                nc.vector.tensor_copy(adjf[ib][:], ai[:])
            for jb in range(NB):
                for half in range(2):
                    tp = psS.tile([128, 512], BF16, tag="ps")
                    for i4 in range(4):
                        ib = half * 4 + i4
                        nc.tensor.transpose(
                            tp[:, i4 * 128:(i4 + 1) * 128],
                            adjf[ib][:, jb * 128:(jb + 1) * 128],
                            identb[:],
                        )
                    nc.vector.tensor_copy(
                        adjT[jb][:, half * 512:(half + 1) * 512], tp[:]
                    )

            # ---- main attention loop ----
            for hh in range(H):
                acc = [psAcc.tile([65, 512], F32, tag=f"acc{c}", name=f"acc{c}")
                       for c in range(2)]
                for jb in range(NB):
                    x1 = x1p.tile([128, N], F32, tag="x1")
                    nc.scalar.activation(
                        x1[:], e_srcb[hh][:], AF.Exp,
                        bias=e_sb[jb][:, 8 + hh:9 + hh],
                    )
                    x2 = x2p.tile([128, N], F32, tag="x2")
                    nc.scalar.activation(
                        x2[:], e_srcb[hh][:], AF.Exp,
                        bias=e_sc[jb][:, 8 + hh:9 + hh], scale=ALPHA,
                    )
                    nc.vector.tensor_max(x1[:], x1[:], x2[:])
                    if jb < GP_MASK_JB:
                        nc.gpsimd.tensor_mul(x1[:], x1[:], adjT[jb][:])
                    else:
                        nc.vector.tensor_mul(x1[:], x1[:], adjT[jb][:])
                    for c in range(2):
                        nc.tensor.matmul(
                            acc[c][:],
                            wh_aug[jb][:, hh * 65:(hh + 1) * 65],
                            x1[:, c * 512:(c + 1) * 512],
                            start=(jb == 0), stop=(jb == NB - 1),
                        )
                # epilogue: copy acc to SBUF (ACT), transpose back, scale
                acc_sb = epi.tile([65, N], F32, tag="accsb")
                for c in range(2):
                    nc.scalar.copy(acc_sb[:, c * 512:(c + 1) * 512], acc[c][:])
                for c in range(NB):
                    tp = psS.tile([128, 512], F32, tag="ps")
                    nc.tensor.transpose(
                        tp[:, 0:65], acc_sb[:, c * 128:(c + 1) * 128],
                        ident[0:65, 0:65],
                    )
                    rec = epi.tile([128, 1], F32, tag="rec")
                    nc.vector.reciprocal(rec[:], tp[:, 64:65])
                    osm = epi.tile([128, FOH], F32, tag="osm", bufs=4)
                    nc.scalar.activation(
                        osm[:], tp[:, 0:64], AF.Copy, scale=rec[:],
                    )
                    nc.sync.dma_start(
                        out_d[c * 128:(c + 1) * 128,
                              hh * FOH:(hh + 1) * FOH], osm[:],
                    )

    if split:
        _split_sync_waits(nc)
    return nc


_NC_CACHE = None


def _get_nc():
    global _NC_CACHE
    if _NC_CACHE is None:
        _NC_CACHE = build_nc()
    return _NC_CACHE


def _prep_in_maps(h, adj, W, a):
    h = np.ascontiguousarray(h, dtype=np.float32)
    adj = np.ascontiguousarray(adj, dtype=np.int32)
    W = np.ascontiguousarray(W, dtype=np.float32)
    a = np.ascontiguousarray(a, dtype=np.float32)
    amat = np.zeros((FO, 2 * H), dtype=np.float32)
    for hh in range(H):
        amat[hh * FOH:(hh + 1) * FOH, hh] = a[hh, :FOH]
        amat[hh * FOH:(hh + 1) * FOH, H + hh] = a[hh, FOH:]
    wamat = (W @ amat).astype(np.float32)
    return [
        {"h": h[c], "adj": adj[c], "W": W, "WA": wamat}
        for c in range(N_CORES)
    ]


def run(h, adj, W, a, trace=False, **kw):
    nc = _get_nc()
    in_maps = _prep_in_maps(h, adj, W, a)
    res = run_bass_kernel_spmd(nc, in_maps, list(range(N_CORES)), trace=trace, **kw)
    out = np.stack([res.results[c]["out"] for c in range(N_CORES)], axis=0)
    return out.astype(np.float32), res


def kernel(h, adj, W, a):
    out, _ = run(h, adj, W, a)
    return out



# revision 4
# speedup vs baseline: 1.7131x; 1.7131x over previous
"""MultiHeadGAT layer on 8 trn2 NeuronCores, data-parallel over batch.

Per core (one batch element), exp(leaky_relu(e_src[i]+e_dst[j])) is
factored rank-1:  with u=exp(e_src), r=exp(-0.8 e_src), v=exp(e_dst),
z=exp(0.2 e_dst):

    exp(lrelu(s_ij)) = u_i * max(r_i z_j, v_j)

The row factor u_i cancels in the softmax, so the per-element work is

    S'[j,i] = adj[i,j] * max(r_i * z_j, v_j)

one fused DVE tensor_scalar (mult+max, fp16 4x mode) + one DVE
tensor_tensor mask multiply (fp16 2x).  No full-size exp at all (exp
only on [8,1024] vectors).  The AV matmul runs fp16 (1 cycle/row) with
a ones column appended to Wh so row 64 of the accumulator is the
softmax denominator.  Epilogue: transpose back 4 row-blocks per PSUM
tile, one strided DVE reciprocal per 4 blocks, ACT scale-copies.
"""
import sys

sys.path.insert(0, "/opt/trn_rl_repo")

import numpy as np

import concourse.bass as bass
import concourse.mybir as mybir
import concourse.tile as tile
from concourse.bass_utils import run_bass_kernel_spmd
from concourse.masks import make_identity

F32 = mybir.dt.float32
FP16 = mybir.dt.float16
I32 = mybir.dt.int32
AF = mybir.ActivationFunctionType
ALU = mybir.AluOpType

N_CORES = 8
N = 1024
NB = 8          # row blocks of 128
FIN = 256
KT = 2          # FIN / 128
FO = 512        # heads * fo
H = 8
FOH = 64
ALPHA = 0.2
NSEL = 4        # heads whose r-broadcast goes via PE selector matmul

# number of mask multiplies (per 64 total) offloaded to gpsimd
GP_MASK = 0

_MAX_SYNC_WAITS = 1


def _split_sync_waits(nc, max_waits=_MAX_SYNC_WAITS):
    """This walrus build rejects instructions carrying more than one sync
    wait; hoist extras onto NOPs inserted just before, on the same engine."""
    uid = 0
    for f in nc.m.functions:
        for bb in f.blocks:
            out = []
            for inst in bb.instructions:
                si = getattr(inst, "sync_info", None)
                if si is not None and si.on_wait and len(si.on_wait) > max_waits:
                    waits = list(si.on_wait)
                    keep = waits[-max_waits:]
                    extra = waits[:-max_waits]
                    si.on_wait.clear()
                    si.on_wait.extend(keep)
                    while extra:
                        chunk, extra = extra[:max_waits], extra[max_waits:]
                        nop = mybir.InstNoOp(
                            name=f"waitsplit-{uid}",
                            engine=inst.engine,
                            sync_info=mybir.SyncInfo(
                                on_wait=list(chunk), on_update=[]
                            ),
                            bass_nofuse=True,
                        )
                        uid += 1
                        out.append(nop)
                out.append(inst)
            bb.instructions[:] = out


def build_nc(split=True):
    nc = bass.Bass()
    h_d = nc.declare_dram_parameter("h", [N, FIN], F32, isOutput=False)
    adj_d = nc.declare_dram_parameter("adj", [N, N], I32, isOutput=False)
    w_d = nc.declare_dram_parameter("W", [FIN, FO], F32, isOutput=False)
    wa_d = nc.declare_dram_parameter("WA", [FIN, 2 * H], F32, isOutput=False)
    out_d = nc.declare_dram_parameter("out", [N, FO], F32, isOutput=True)

    with tile.TileContext(nc) as tc:
        with (
            tc.tile_pool(name="const", bufs=1) as const,
            tc.tile_pool(name="persist", bufs=1) as persist,
            tc.tile_pool(name="ld", bufs=4) as ld,
            tc.tile_pool(name="xp", bufs=6) as xp,
            tc.tile_pool(name="epi", bufs=2) as epi,
            tc.tile_pool(name="psS", bufs=2, space="PSUM") as psS,
            tc.tile_pool(name="psAcc", bufs=2, space="PSUM") as psAcc,
        ):
            ident = const.tile([128, 128], F32, tag="ident")
            make_identity(nc, ident[:])
            identh = const.tile([128, 128], FP16, tag="identh")
            nc.vector.tensor_copy(identh[:], ident[:])

            # ---- adj: int32 -> fp16 cast DMA (SWDGE), then PE transpose ----
            adjf = [persist.tile([128, N], FP16, tag=f"adjf{i}", name=f"adjf{i}")
                    for i in range(NB)]
            for ib in range(NB):
                nc.gpsimd.dma_start(
                    adjf[ib][:], adj_d[ib * 128:(ib + 1) * 128, :]
                )

            # ---- weights (fp32 load -> fp16 cast) ----
            wk = []
            for k in range(KT):
                t32 = ld.tile([128, FO], F32, tag="w32", name=f"w32_{k}")
                nc.sync.dma_start(t32[:], w_d[k * 128:(k + 1) * 128, :])
                t = const.tile([128, FO], FP16, tag=f"W{k}", name=f"W{k}")
                nc.scalar.copy(t[:], t32[:])
                wk.append(t)
            wa = []
            for k in range(KT):
                t32 = ld.tile([128, 2 * H], F32, tag="wa32", name=f"wa32_{k}")
                nc.sync.dma_start(t32[:], wa_d[k * 128:(k + 1) * 128, :])
                t = const.tile([128, 2 * H], FP16, tag=f"WA{k}", name=f"WA{k}")
                nc.scalar.copy(t[:], t32[:])
                wa.append(t)

            # ---- hT[k][f128, i] = h[i, k*128+f]  (fp16) ----
            hT = [persist.tile([128, N], FP16, tag=f"hT{k}", name=f"hT{k}")
                  for k in range(KT)]
            for ibq in range(2):      # groups of 4 row-blocks
                hts = []
                for i4 in range(4):
                    ib = ibq * 4 + i4
                    ht = ld.tile([128, FIN], F32, tag="hld")
                    nc.sync.dma_start(ht[:], h_d[ib * 128:(ib + 1) * 128, :])
                    hts.append(ht)
                for k in range(KT):
                    tp = psS.tile([128, 512], F32, tag="ps")
                    for i4 in range(4):
                        nc.tensor.transpose(
                            tp[:, i4 * 128:(i4 + 1) * 128],
                            hts[i4][:, k * 128:(k + 1) * 128], ident[:],
                        )
                    nc.scalar.copy(
                        hT[k][:, ibq * 512:(ibq + 1) * 512], tp[:]
                    )

            # ---- e_src_t[8, i], e_dst_t[8, i] = (WA.T @ hT) halves ----
            e_src_t = const.tile([8, N], F32, tag="esT")
            e_dst_t = const.tile([8, N], F32, tag="edT")
            for c in range(2):
                for half, dst in ((0, e_src_t), (1, e_dst_t)):
                    ps = psS.tile([8, 512], F32, tag="ps")
                    for k in range(KT):
                        nc.tensor.matmul(
                            ps[:], wa[k][:, half * 8:(half + 1) * 8],
                            hT[k][:, c * 512:(c + 1) * 512],
                            start=(k == 0), stop=(k == KT - 1),
                        )
                    nc.vector.tensor_copy(dst[:, c * 512:(c + 1) * 512], ps[:])

            # ---- derived exp vectors ----
            # rv_t[hh, i] = exp(-0.8 * e_src[hh, i])        (fp16)
            rv_t = const.tile([8, N], FP16, tag="rvT")
            nc.scalar.activation(rv_t[:], e_src_t[:], AF.Exp, scale=-0.8)
            # v = exp(e_dst); z = exp(0.2 e_dst)
            v_t = const.tile([8, N], F32, tag="vT")
            z_t = const.tile([8, N], F32, tag="zT")
            nc.scalar.activation(v_t[:], e_dst_t[:], AF.Exp)
            nc.scalar.activation(z_t[:], e_dst_t[:], AF.Exp, scale=ALPHA)

            # ---- vz_sb[jb][p, 0:8]=v_h(j), [p, 8:16]=z_h(j) ----
            vz_sb = [persist.tile([128, 16], F32, tag=f"vz{j}", name=f"vz{j}")
                     for j in range(NB)]
            for jb in range(NB):
                tp = psS.tile([128, 512], F32, tag="ps")
                nc.tensor.transpose(
                    tp[:, 0:8], v_t[:, jb * 128:(jb + 1) * 128],
                    ident[0:8, 0:8],
                )
                nc.tensor.transpose(
                    tp[:, 8:16], z_t[:, jb * 128:(jb + 1) * 128],
                    ident[0:8, 0:8],
                )
                nc.vector.tensor_copy(vz_sb[jb][:], tp[:, 0:16])

            # ---- r_all[p, hh*N + i] = rv_t[hh, i] broadcast over partitions.
            # Heads 0..NSEL-1 via PE selector matmul (low latency); the rest
            # via DMA log-doubling (hidden behind the first heads). ----
            r_all = persist.tile([128, H * N], FP16, tag="rall")
            sel = []
            for hh in range(NSEL):
                t = const.tile([8, 128], FP16, tag=f"sel{hh}", name=f"sel{hh}")
                nc.gpsimd.memset(t[:], 0.0)
                nc.gpsimd.affine_select(
                    out=t[:], in_=t[:], pattern=[[0, 128]],
                    compare_op=ALU.not_equal, fill=1.0,
                    base=-hh, channel_multiplier=1,
                )
                sel.append(t)
            for hh in range(NSEL):
                for c in range(2):
                    ps = psS.tile([128, 512], F32, tag="ps")
                    nc.tensor.matmul(
                        ps[:], sel[hh][:], rv_t[:, c * 512:(c + 1) * 512],
                        start=True, stop=True,
                    )
                    nc.scalar.copy(
                        r_all[:, hh * N + c * 512:hh * N + (c + 1) * 512],
                        ps[:],
                    )
            if NSEL < H:
                nc.sync.dma_start(
                    r_all[0:1, NSEL * N:H * N], rv_t[NSEL:H, :]
                )
                p = 1
                while p < 128:
                    nc.sync.dma_start(
                        r_all[p:2 * p, NSEL * N:H * N],
                        r_all[0:p, NSEL * N:H * N],
                    )
                    p *= 2

            # ---- Wh_aug[jb][:, hh*65:+64] = (h @ W) block fp16, col 64 = 1 ----
            wh_aug = [persist.tile([128, H * 65], FP16, tag=f"wha{j}",
                                   name=f"wha{j}")
                      for j in range(NB)]
            for jb in range(NB):
                ps = psS.tile([128, 512], F32, tag="ps")
                for k in range(KT):
                    nc.tensor.matmul(
                        ps[:], hT[k][:, jb * 128:(jb + 1) * 128], wk[k][:],
                        start=(k == 0), stop=(k == KT - 1),
                    )
                wv = wh_aug[jb][:].rearrange("p (h f) -> p h f", h=H)
                pv = ps[:].rearrange("p (h f) -> p h f", h=H)
                nc.scalar.copy(wv[:, :, 0:64], pv[:])
                nc.gpsimd.memset(wv[:, :, 64:65], 1.0)

            # ---- adjT[jb][j128, i] = adj[i, jb*128+j]  (fp16, PE transpose) ----
            adjT = [persist.tile([128, N], FP16, tag=f"adjT{j}",
                                 name=f"adjT{j}")
                    for j in range(NB)]
            for jb in range(NB):
                for half in range(2):
                    tp = psS.tile([128, 512], FP16, tag="ps")
                    for i4 in range(4):
                        ib = half * 4 + i4
                        nc.tensor.transpose(
                            tp[:, i4 * 128:(i4 + 1) * 128],
                            adjf[ib][:, jb * 128:(jb + 1) * 128],
                            identh[:],
                        )
                    nc.scalar.copy(
                        adjT[jb][:, half * 512:(half + 1) * 512], tp[:]
                    )

            # ---- out staging: out_sb[ib][:, hh*64+f], DMA'd once per ib ----
            out_sb = [persist.tile([128, FO], F32, tag=f"os{i}", name=f"os{i}")
                      for i in range(NB)]

            # ---- main attention loop ----
            for hh in range(H):
                acc = [psAcc.tile([65, 512], F32, tag=f"acc{c}",
                                  name=f"acc{c}")
                       for c in range(2)]
                for jb in range(NB):
                    x = xp.tile([128, N], FP16, tag="x")
                    nc.vector.tensor_scalar(
                        x[:], r_all[:, hh * N:(hh + 1) * N],
                        vz_sb[jb][:, 8 + hh:9 + hh],
                        vz_sb[jb][:, hh:hh + 1],
                        ALU.mult, ALU.max,
                    )
                    if jb < GP_MASK:
                        nc.gpsimd.tensor_mul(x[:], x[:], adjT[jb][:])
                    else:
                        nc.vector.tensor_mul(x[:], x[:], adjT[jb][:])
                    for c in range(2):
                        nc.tensor.matmul(
                            acc[c][:],
                            wh_aug[jb][:, hh * 65:(hh + 1) * 65],
                            x[:, c * 512:(c + 1) * 512],
                            start=(jb == 0), stop=(jb == NB - 1),
                        )
                # epilogue: PSUM->SBUF (ACT), transpose back 4 blocks per
                # PSUM tile, strided reciprocal, ACT scale-copies.
                acc_sb = epi.tile([65, N], F32, tag="accsb")
                for c in range(2):
                    nc.scalar.copy(acc_sb[:, c * 512:(c + 1) * 512], acc[c][:])
                for half in range(2):
                    tp4 = psS.tile([128, 260], F32, tag="tp4", bufs=2)
                    t4v = tp4[:].rearrange("p (q f) -> p q f", f=65)
                    for q in range(4):
                        ib = half * 4 + q
                        nc.tensor.transpose(
                            tp4[:, q * 65:(q + 1) * 65],
                            acc_sb[:, ib * 128:(ib + 1) * 128],
                            ident[0:65, 0:65],
                        )
                    rec4 = epi.tile([128, 4], F32, tag="rec4", bufs=3)
                    r4v = rec4[:].rearrange("p (q o) -> p q o", o=1)
                    nc.vector.reciprocal(r4v[:], t4v[:, :, 64:65])
                    for q in range(4):
                        ib = half * 4 + q
                        nc.scalar.activation(
                            out_sb[ib][:, hh * FOH:(hh + 1) * FOH],
                            tp4[:, q * 65:q * 65 + 64],
                            AF.Copy, scale=rec4[:, q:q + 1],
                        )
                        if hh == H - 1:
                            nc.sync.dma_start(
                                out_d[ib * 128:(ib + 1) * 128, :],
                                out_sb[ib][:],
                            )

    if split:
        _split_sync_waits(nc)
    return nc


_NC_CACHE = None


def _get_nc():
    global _NC_CACHE
    if _NC_CACHE is None:
        _NC_CACHE = build_nc()
    return _NC_CACHE


def _prep_in_maps(h, adj, W, a):
    h = np.ascontiguousarray(h, dtype=np.float32)
    adj = np.ascontiguousarray(adj, dtype=np.int32)
    W = np.ascontiguousarray(W, dtype=np.float32)
    a = np.ascontiguousarray(a, dtype=np.float32)
    amat = np.zeros((FO, 2 * H), dtype=np.float32)
    for hh in range(H):
        amat[hh * FOH:(hh + 1) * FOH, hh] = a[hh, :FOH]
        amat[hh * FOH:(hh + 1) * FOH, H + hh] = a[hh, FOH:]
    wamat = (W @ amat).astype(np.float32)
    return [
        {"h": h[c], "adj": adj[c], "W": W, "WA": wamat}
        for c in range(N_CORES)
    ]


def run(h, adj, W, a, trace=False, **kw):
    nc = _get_nc()
    in_maps = _prep_in_maps(h, adj, W, a)
    res = run_bass_kernel_spmd(nc, in_maps, list(range(N_CORES)), trace=trace, **kw)
    out = np.stack([res.results[c]["out"] for c in range(N_CORES)], axis=0)
    return out.astype(np.float32), res


def kernel(h, adj, W, a):
    out, _ = run(h, adj, W, a)
    return out
